# revision 14
# baseline (speedup 1.0000x reference)
"""Trainium2 Bass kernel for nn_Coefficients: assemble the sparse circuit
coefficient matrix

    out = [ kcl  = [ M | 0 ]                       (N rows)
            kvl  = [ 0 | I_E | -M^T ]              (E rows)
            elem = diag(z) / diag(y) scatter ]     (E rows)

Device work (per core d, which owns M row-shard M[d*256:(d+1)*256, :]):
  - mtc:  -shard^T = a 256-column slice of the kvl -M^T block,
          produced by PE transpose-mode -> full-bank PSUM -> negating
          DVE copy -> SBUF chunk -> incremental DMA
  - zyo:  per-element diagonal VALUES (z diag, y diag, I ones; one
          [128,12] f32 write) computed from params/kinds on GpSimd.

Host side is pure assembly/indexing: the 97%-zero canvas, the kcl block
(out[0:N, 0:E] = M -- a verbatim copy of the input, so routing it
through the device would be pure excess HBM traffic), the diagonal
scatter of zyo values, and the unscramble of the mtc layout.

Data moves as bf16 (correctness gate is rel_err < 2e-2; bf16
round-to-nearest gives ~3e-3). Per-core DMA transfer bytes: 2.1 in +
2.1 out = 4.2 MB vs 6.3 MB when kcl is echoed through the device.

Queue layout (each HWDGE ring sustains ~195 GB/s; 16 SDMA engines are
shared; per-core aggregate ~358 GB/s):
  - sync/Q1:    g=0 chunk loads, then 3 of 8 mtc group writes
  - scalar/Q10: g=1 chunk loads, then 3 of 8 mtc group writes
  - gpsimd/Q0:  the 2 EARLIEST-ready mtc groups (they dispatch while
    the rings are still busy with chunk1 loads) + the zyo write

params/kinds ride as 8 extra bf16 COLUMNS of the m tensor (cols
4096:4104, replicated per row-group) so no tiny-descriptor input DMAs
exist; the z/y math reads them as views of the last chunk tile.

mtc device layout [128, 8192]: mtc[p, g*4096 + cb*128 + j] =
-M[128g + j, 128cb + p] (g = row-group, cb = column-block). Host
unscrambles with one reshape/transpose - pure indexing.

Notes from measurement (do not redo):
  - small lead chunks: WORSE (DMA completion semaphores have ~2-3us
    fixed latency; small loads don't unblock the PE proportionally
    earlier and wreck the ramp)
  - dependency-free DRAM->DRAM kcl copy on Q0: WORSE (SDMA round-robin
    is per-packet; 8KB descriptors starve the 4KB load descriptors)
"""

import numpy as np

N = 2048
E = 4096
W = 2 * E + N  # 10240
D = 8
NR = N // D  # 256 kcl rows / mt cols per core
EC = E // D  # 512 elem rows per core
EXT = 8  # extra m columns carrying params (4) + kinds (4)

_CACHE: dict = {}


def _build(opts=None):
    import concourse.bacc as bacc
    import concourse.tile as tile
    import concourse.mybir as mybir
    from concourse._compat import get_trn_type

    opts = dict(opts or {})
    ppool_bufs = opts.get("ppool_bufs", 8)
    use_bf16 = opts.get("dtype", "bf16") == "bf16"
    part_id = opts.get("partition_id", False)
    WS = list(opts.get("ws", (2048, 2048)))
    assert sum(WS) == E
    # mtc write-group engine assignment in readiness order
    # (0=sync ring, 1=scalar ring, 2=gpsimd SWDGE). Q0 (SWDGE) is slow
    # (~100 GB/s): give it only the EARLIEST group, never a late one.
    MTC_ENG = list(opts.get("mtc_eng", (2, 0, 1, 2, 0, 1, 0, 1)))

    f32 = mybir.dt.float32
    mdt = mybir.dt.bfloat16 if use_bf16 else f32

    nc = bacc.Bacc(
        get_trn_type() or "TRN2",
        target_bir_lowering=False,
        debug=False,
        enable_asserts=False,
        num_devices=D,
        enable_partition_id=part_id,
    )

    m = nc.dram_tensor("m", [NR, E], mdt, kind="ExternalInput")
    pkt = nc.dram_tensor("pk", [128, EXT], f32, kind="ExternalInput")

    mtc = nc.dram_tensor("mtc", [128, 2 * E], mdt, kind="ExternalOutput")
    zyo = nc.dram_tensor("zyo", [128, 12], f32, kind="ExternalOutput")

    AO = mybir.AluOpType
    NCH = len(WS)
    CS = [sum(WS[:i]) for i in range(NCH)]  # chunk column starts
    PSW = 1024 if use_bf16 else 512  # full 2KB-per-partition psum bank

    def psum_groups(w):
        # split a chunk width into PSW-sized groups + one remainder
        offs, o = [], 0
        while o < w:
            g = min(PSW, w - o)
            offs.append((o, g))
            o += g
        return offs

    with tile.TileContext(nc) as tc:
        with (
            tc.tile_pool(name="cpool", bufs=1) as cpool,
            tc.tile_pool(name="ppool", bufs=ppool_bufs, space="PSUM") as ppool,
        ):
            # ---- identity for PE transpose-mode, FIRST on gpsimd (PE dep)
            ident = cpool.tile([128, 128], mdt)
            nc.gpsimd.memset(ident[:], 0.0)
            nc.gpsimd.affine_select(
                out=ident[:],
                in_=ident[:],
                compare_op=AO.not_equal,
                fill=1.0,
                base=0,
                pattern=[[-1, 128]],
                channel_multiplier=1,
            )

            # ---- params/kinds: tiny [128, EXT] f32 load on the otherwise-
            # idle Q0 (SWDGE), dispatched at kernel start. Even with the
            # ~2.5us SWDGE first-dispatch latency it lands ~10.5us, well
            # before the rings would deliver it. Keeping it out of `m` also
            # keeps every ring descriptor a clean 4096B (a 4112B row splits
            # into a 4096B packet + a 16B RUNT packet per row, and 128 runts
            # measurably clog the ring feed).
            pk = cpool.tile([128, EXT], f32)
            nc.gpsimd.dma_start(out=pk[:], in_=pkt.ap()[:, :])

            # ---- M row-shard chunk loads on the HWDGE rings (g -> ring)
            mch = [[None] * NCH for _ in range(2)]
            for ci in range(NCH):
                w = WS[ci]
                for g in range(2):
                    t = cpool.tile([128, w], mdt, tag=f"m{g}{ci}")
                    eng = nc.sync if g == 0 else nc.scalar
                    eng.dma_start(
                        out=t[:],
                        in_=m.ap()[g * 128 : (g + 1) * 128, CS[ci] : CS[ci] + w],
                    )
                    mch[g][ci] = t

            # ---- diagonal values on GpSimd (emitted BEFORE the mtc loop so
            # the chain runs as soon as pk lands, ~10.5us, and zyo's Q0
            # dispatch precedes the Q0 mtc-group dispatch). Rings, PE, DVE
            # untouched by any of this.
            pv = pk[:, 0:4]  # params
            kv = pk[:, 4:8]  # kinds

            zy = cpool.tile([128, 12], f32)
            nc.gpsimd.memset(zy[:, 8:12], 1.0)  # I_E diag ones
            # one backing tile for all temporaries (fewer tile semaphores
            # -> shorter end-of-kernel semaphore-clear chain); the chain is
            # serial anyway so the false intra-tile deps are harmless
            tmp = cpool.tile([128, 36], f32)
            pf = tmp[:, 0:4]
            rm = tmp[:, 4:8]
            im = tmp[:, 8:12]
            vm = tmp[:, 12:16]
            sm = tmp[:, 16:20]
            onm = tmp[:, 20:24]
            offm = tmp[:, 24:28]
            t0 = tmp[:, 28:32]
            t1 = tmp[:, 32:36]

            nc.gpsimd.tensor_scalar(pf, pv, 1.0, None, op0=AO.mult)
            nc.gpsimd.tensor_scalar(rm, kv, 0.0, None, op0=AO.is_equal)
            nc.gpsimd.tensor_scalar(im, kv, 1.0, None, op0=AO.is_equal)
            nc.gpsimd.tensor_scalar(vm, kv, 2.0, None, op0=AO.is_equal)
            nc.gpsimd.tensor_scalar(sm, kv, 3.0, None, op0=AO.is_equal)
            nc.gpsimd.tensor_scalar(onm, pf, 0.0, None, op0=AO.is_gt)
            nc.gpsimd.tensor_scalar(offm, pf, 0.0, None, op0=AO.is_le)
            # z = vc + sw*off - r*params
            nc.gpsimd.tensor_tensor(t0, sm, offm, op=AO.mult)
            nc.gpsimd.tensor_tensor(t0, vm, t0, op=AO.add)
            nc.gpsimd.tensor_tensor(t1, rm, pf, op=AO.mult)
            nc.gpsimd.tensor_tensor(zy[:, 0:4], t0, t1, op=AO.subtract)
            # y = r + ivs + sw*on
            nc.gpsimd.tensor_tensor(t0, sm, onm, op=AO.mult)
            nc.gpsimd.tensor_tensor(t0, im, t0, op=AO.add)
            nc.gpsimd.tensor_tensor(zy[:, 4:8], rm, t0, op=AO.add)
            nc.gpsimd.dma_start(out=zyo.ap()[:, :], in_=zy[:])

            # ---- -M^T column slice: PE transposes chunks as they land; DVE
            # drains psum banks with negation into staging tiles; every psum
            # group DMAs out immediately on its assigned queue (MTC_ENG in
            # readiness order).
            ENGS = [nc.sync, nc.scalar, nc.gpsimd]
            mgrp = 0
            for ci in range(NCH):
                for g in range(2):
                    for o, w in psum_groups(WS[ci]):
                        ps = ppool.tile([128, w], mdt)
                        for jj in range(w // 128):
                            lo = o + jj * 128
                            nc.tensor.transpose(
                                out=ps[:, jj * 128 : (jj + 1) * 128],
                                in_=mch[g][ci][:, lo : lo + 128],
                                identity=ident[:],
                            )
                        mt_st = cpool.tile([128, w], mdt, tag=f"t{g}{ci}{o}")
                        nc.vector.tensor_scalar(
                            mt_st[:], ps[:], -1.0, None, op0=AO.mult
                        )
                        eng = ENGS[MTC_ENG[mgrp % len(MTC_ENG)]]
                        mgrp += 1
                        f0 = g * E + CS[ci] + o
                        eng.dma_start(out=mtc.ap()[:, f0 : f0 + w], in_=mt_st[:])

    nc.compile()
    return nc


def _get_nc(opts=None):
    key = ("nc", tuple(sorted((opts or {}).items())))
    if key not in _CACHE:
        _CACHE[key] = _build(opts)
    return _CACHE[key]


def _in_maps(M, params, kinds, use_bf16):
    if use_bf16:
        import ml_dtypes

        dt = ml_dtypes.bfloat16
    else:
        dt = np.float32
    maps = []
    for d in range(D):
        pk = np.empty((128, EXT), dtype=np.float32)
        pk[:, 0:4] = params[d * EC : (d + 1) * EC].reshape(4, 128).T
        pk[:, 4:8] = kinds[d * EC : (d + 1) * EC].reshape(4, 128).T
        maps.append({"m": M[d * NR : (d + 1) * NR, :].astype(dt), "pk": pk})
    return maps


def kernel(M, params, kinds, _trace=False, _trace_kwargs=None, _opts=None):
    from concourse.bass_utils import run_bass_kernel_spmd

    M = np.ascontiguousarray(np.asarray(M, dtype=np.float32))
    params = np.ascontiguousarray(np.asarray(params, dtype=np.float32))
    kinds = np.ascontiguousarray(np.asarray(kinds, dtype=np.int32))
    assert M.shape == (N, E) and params.shape == (E,) and kinds.shape == (E,)

    opts = dict(_opts or {})
    use_bf16 = opts.get("dtype", "bf16") == "bf16"
    nc = _get_nc(opts)
    res = run_bass_kernel_spmd(
        nc,
        _in_maps(M, params, kinds, use_bf16),
        core_ids=list(range(D)),
        trace=_trace,
        **(_trace_kwargs or {}),
    )
    out = np.zeros((N + 2 * E, W), np.float32)
    # kcl block: out[0:N, 0:E] = M verbatim (host-side copy of the input;
    # no device round-trip)
    out[0:N, 0:E] = M
    for d in range(D):
        r = res.results[d]
        # kvl -M^T block: column slice [E, 256] for this core's nodes.
        # mtc[p, g*4096 + cb*128 + j] = -M[128g+j, 128cb+p]
        v = np.asarray(r["mtc"]).reshape(128, 2, 32, 128)
        mts = v.transpose(2, 0, 1, 3).reshape(E, NR)
        out[N : N + E, 2 * E + d * NR : 2 * E + (d + 1) * NR] = mts
        # diagonals: zyo = [z | y | ones], value layout r = c*128 + p
        gs = d * EC + np.arange(EC)
        zy = r["zyo"]
        z_flat = zy[:, 0:4].T.reshape(EC)
        y_flat = zy[:, 4:8].T.reshape(EC)
        o_flat = zy[:, 8:12].T.reshape(EC)
        out[N + gs, E + gs] = o_flat  # I_E diag in kvl rows
        out[N + E + gs, gs] = z_flat  # elem z diag
        out[N + E + gs, E + gs] = y_flat  # elem y diag
    if _trace:
        _CACHE["last_result"] = res
    return out


# revision 17
# speedup vs baseline: 1.0263x; 1.0263x over previous
"""Trainium2 Bass kernel for nn_Coefficients: assemble the sparse circuit
coefficient matrix

    out = [ kcl  = [ M | 0 ]                       (N rows)
            kvl  = [ 0 | I_E | -M^T ]              (E rows)
            elem = diag(z) / diag(y) scatter ]     (E rows)

Device work (per core d, which owns M row-shard M[d*256:(d+1)*256, :]):
  - mtc:  -shard^T = a 256-column slice of the kvl -M^T block,
          produced by PE transpose-mode -> full-bank PSUM -> negating
          DVE copy -> SBUF chunk -> incremental DMA
  - zyo:  per-element diagonal VALUES (z diag, y diag, I ones; one
          [128,12] f32 write) computed from params/kinds on GpSimd.

Host side is pure assembly/indexing: the 97%-zero canvas, the kcl block
(out[0:N, 0:E] = M -- a verbatim copy of the input, so routing it
through the device would be pure excess HBM traffic), the diagonal
scatter of zyo values, and the unscramble of the mtc layout.

Data moves as bf16 (correctness gate is rel_err < 2e-2; bf16
round-to-nearest gives ~3e-3). Per-core DMA transfer bytes: 2.1 in +
2.1 out = 4.2 MB vs 6.3 MB when kcl is echoed through the device.

Queue layout (each HWDGE ring sustains ~195 GB/s; 16 SDMA engines are
shared; per-core aggregate ~358 GB/s):
  - sync/Q1:    g=0 chunk loads, then 3 of 8 mtc group writes
  - scalar/Q10: g=1 chunk loads, then 3 of 8 mtc group writes
  - gpsimd/Q0:  the 2 EARLIEST-ready mtc groups (they dispatch while
    the rings are still busy with chunk1 loads) + the zyo write

params/kinds ride as 8 extra bf16 COLUMNS of the m tensor (cols
4096:4104, replicated per row-group) so no tiny-descriptor input DMAs
exist; the z/y math reads them as views of the last chunk tile.

mtc device layout [128, 8192]: mtc[p, g*4096 + cb*128 + j] =
-M[128g + j, 128cb + p] (g = row-group, cb = column-block). Host
unscrambles with one reshape/transpose - pure indexing.

Notes from measurement (do not redo):
  - small lead chunks: WORSE (DMA completion semaphores have ~2-3us
    fixed latency; small loads don't unblock the PE proportionally
    earlier and wreck the ramp)
  - dependency-free DRAM->DRAM kcl copy on Q0: WORSE (SDMA round-robin
    is per-packet; 8KB descriptors starve the 4KB load descriptors)
"""

import numpy as np

N = 2048
E = 4096
W = 2 * E + N  # 10240
D = 8
NR = N // D  # 256 kcl rows / mt cols per core
EC = E // D  # 512 elem rows per core
EXT = 8  # extra m columns carrying params (4) + kinds (4)

_CACHE: dict = {}


def _build(opts=None):
    import concourse.bacc as bacc
    import concourse.tile as tile
    import concourse.mybir as mybir
    from concourse._compat import get_trn_type

    opts = dict(opts or {})
    ppool_bufs = opts.get("ppool_bufs", 8)
    use_bf16 = opts.get("dtype", "bf16") == "bf16"
    part_id = opts.get("partition_id", False)
    WS = list(opts.get("ws", (2048, 2048)))
    assert sum(WS) == E
    # mtc write-group engine assignment in readiness order
    # (0=sync ring, 1=scalar ring, 2=gpsimd SWDGE). Q0 (SWDGE) is slow
    # (~100 GB/s): give it only the EARLIEST group, never a late one.
    MTC_ENG = list(opts.get("mtc_eng", (2, 0, 1, 2, 0, 1, 0, 1)))

    f32 = mybir.dt.float32
    mdt = mybir.dt.bfloat16 if use_bf16 else f32

    nc = bacc.Bacc(
        get_trn_type() or "TRN2",
        target_bir_lowering=False,
        debug=False,
        enable_asserts=False,
        num_devices=D,
        enable_partition_id=part_id,
    )

    m = nc.dram_tensor("m", [NR, E], mdt, kind="ExternalInput")
    pkt = nc.dram_tensor("pk", [128, EXT], f32, kind="ExternalInput")

    mtc = nc.dram_tensor("mtc", [128, 2 * E], mdt, kind="ExternalOutput")
    zyo = nc.dram_tensor("zyo", [128, 12], f32, kind="ExternalOutput")

    AO = mybir.AluOpType
    NCH = len(WS)
    CS = [sum(WS[:i]) for i in range(NCH)]  # chunk column starts
    PSW = 1024 if use_bf16 else 512  # full 2KB-per-partition psum bank

    def psum_groups(w):
        # split a chunk width into PSW-sized groups + one remainder
        offs, o = [], 0
        while o < w:
            g = min(PSW, w - o)
            offs.append((o, g))
            o += g
        return offs

    with tile.TileContext(nc) as tc:
        with (
            tc.tile_pool(name="cpool", bufs=1) as cpool,
            tc.tile_pool(name="ppool", bufs=ppool_bufs, space="PSUM") as ppool,
        ):
            # ---- identity for PE transpose-mode, FIRST on gpsimd (PE dep)
            ident = cpool.tile([128, 128], mdt)
            nc.gpsimd.memset(ident[:], 0.0)
            nc.gpsimd.affine_select(
                out=ident[:],
                in_=ident[:],
                compare_op=AO.not_equal,
                fill=1.0,
                base=0,
                pattern=[[-1, 128]],
                channel_multiplier=1,
            )

            # ---- params/kinds: tiny [128, EXT] f32 load on the otherwise-
            # idle Q0 (SWDGE), dispatched at kernel start. Even with the
            # ~2.5us SWDGE first-dispatch latency it lands ~10.5us, well
            # before the rings would deliver it. Keeping it out of `m` also
            # keeps every ring descriptor a clean 4096B (a 4112B row splits
            # into a 4096B packet + a 16B RUNT packet per row, and 128 runts
            # measurably clog the ring feed).
            pk = cpool.tile([128, EXT], f32)
            nc.gpsimd.dma_start(out=pk[:], in_=pkt.ap()[:, :])

            # ---- M row-shard chunk loads on the HWDGE rings, assigned by
            # CHUNK (sync ring: all chunk-0 tiles, scalar ring: all chunk-1
            # tiles). The scalar ring (Q10) consistently starts ~2-3.5us
            # after Q1; chunk-major assignment parks the late-needed chunks
            # on it so its startup lag hides behind PE's work on chunk 0.
            mch = [[None] * NCH for _ in range(2)]
            for ci in range(NCH):
                w = WS[ci]
                eng = nc.sync if ci < (NCH + 1) // 2 else nc.scalar
                for g in range(2):
                    t = cpool.tile([128, w], mdt, tag=f"m{g}{ci}")
                    eng.dma_start(
                        out=t[:],
                        in_=m.ap()[g * 128 : (g + 1) * 128, CS[ci] : CS[ci] + w],
                    )
                    mch[g][ci] = t

            # ---- diagonal values on GpSimd (emitted BEFORE the mtc loop so
            # the chain runs as soon as pk lands, ~10.5us, and zyo's Q0
            # dispatch precedes the Q0 mtc-group dispatch). Rings, PE, DVE
            # untouched by any of this.
            pv = pk[:, 0:4]  # params
            kv = pk[:, 4:8]  # kinds

            zy = cpool.tile([128, 12], f32)
            nc.gpsimd.memset(zy[:, 8:12], 1.0)  # I_E diag ones
            # one backing tile for all temporaries (fewer tile semaphores
            # -> shorter end-of-kernel semaphore-clear chain); the chain is
            # serial anyway so the false intra-tile deps are harmless
            tmp = cpool.tile([128, 36], f32)
            pf = tmp[:, 0:4]
            rm = tmp[:, 4:8]
            im = tmp[:, 8:12]
            vm = tmp[:, 12:16]
            sm = tmp[:, 16:20]
            onm = tmp[:, 20:24]
            offm = tmp[:, 24:28]
            t0 = tmp[:, 28:32]
            t1 = tmp[:, 32:36]

            nc.gpsimd.tensor_scalar(pf, pv, 1.0, None, op0=AO.mult)
            nc.gpsimd.tensor_scalar(rm, kv, 0.0, None, op0=AO.is_equal)
            nc.gpsimd.tensor_scalar(im, kv, 1.0, None, op0=AO.is_equal)
            nc.gpsimd.tensor_scalar(vm, kv, 2.0, None, op0=AO.is_equal)
            nc.gpsimd.tensor_scalar(sm, kv, 3.0, None, op0=AO.is_equal)
            nc.gpsimd.tensor_scalar(onm, pf, 0.0, None, op0=AO.is_gt)
            nc.gpsimd.tensor_scalar(offm, pf, 0.0, None, op0=AO.is_le)
            # z = vc + sw*off - r*params
            nc.gpsimd.tensor_tensor(t0, sm, offm, op=AO.mult)
            nc.gpsimd.tensor_tensor(t0, vm, t0, op=AO.add)
            nc.gpsimd.tensor_tensor(t1, rm, pf, op=AO.mult)
            nc.gpsimd.tensor_tensor(zy[:, 0:4], t0, t1, op=AO.subtract)
            # y = r + ivs + sw*on
            nc.gpsimd.tensor_tensor(t0, sm, onm, op=AO.mult)
            nc.gpsimd.tensor_tensor(t0, im, t0, op=AO.add)
            nc.gpsimd.tensor_tensor(zy[:, 4:8], rm, t0, op=AO.add)
            nc.gpsimd.dma_start(out=zyo.ap()[:, :], in_=zy[:])

            # ---- -M^T column slice: PE transposes chunks as they land; DVE
            # drains psum banks with negation into staging tiles; every psum
            # group DMAs out immediately on its assigned queue (MTC_ENG in
            # readiness order).
            ENGS = [nc.sync, nc.scalar, nc.gpsimd]
            mgrp = 0
            for ci in range(NCH):
                for g in range(2):
                    for o, w in psum_groups(WS[ci]):
                        ps = ppool.tile([128, w], mdt)
                        for jj in range(w // 128):
                            lo = o + jj * 128
                            nc.tensor.transpose(
                                out=ps[:, jj * 128 : (jj + 1) * 128],
                                in_=mch[g][ci][:, lo : lo + 128],
                                identity=ident[:],
                            )
                        mt_st = cpool.tile([128, w], mdt, tag=f"t{g}{ci}{o}")
                        # drains alternate DVE/ACT so the drain chain never
                        # lags the PE group cadence
                        if mgrp % 2 == 0:
                            nc.vector.tensor_scalar(
                                mt_st[:], ps[:], -1.0, None, op0=AO.mult
                            )
                        else:
                            nc.scalar.mul(mt_st[:], ps[:], -1.0)
                        eng = ENGS[MTC_ENG[mgrp % len(MTC_ENG)]]
                        mgrp += 1
                        f0 = g * E + CS[ci] + o
                        eng.dma_start(out=mtc.ap()[:, f0 : f0 + w], in_=mt_st[:])

    nc.compile()
    return nc


def _get_nc(opts=None):
    key = ("nc", tuple(sorted((opts or {}).items())))
    if key not in _CACHE:
        _CACHE[key] = _build(opts)
    return _CACHE[key]


def _in_maps(M, params, kinds, use_bf16):
    if use_bf16:
        import ml_dtypes

        dt = ml_dtypes.bfloat16
    else:
        dt = np.float32
    maps = []
    for d in range(D):
        pk = np.empty((128, EXT), dtype=np.float32)
        pk[:, 0:4] = params[d * EC : (d + 1) * EC].reshape(4, 128).T
        pk[:, 4:8] = kinds[d * EC : (d + 1) * EC].reshape(4, 128).T
        maps.append({"m": M[d * NR : (d + 1) * NR, :].astype(dt), "pk": pk})
    return maps


def kernel(M, params, kinds, _trace=False, _trace_kwargs=None, _opts=None):
    from concourse.bass_utils import run_bass_kernel_spmd

    M = np.ascontiguousarray(np.asarray(M, dtype=np.float32))
    params = np.ascontiguousarray(np.asarray(params, dtype=np.float32))
    kinds = np.ascontiguousarray(np.asarray(kinds, dtype=np.int32))
    assert M.shape == (N, E) and params.shape == (E,) and kinds.shape == (E,)

    opts = dict(_opts or {})
    use_bf16 = opts.get("dtype", "bf16") == "bf16"
    nc = _get_nc(opts)
    res = run_bass_kernel_spmd(
        nc,
        _in_maps(M, params, kinds, use_bf16),
        core_ids=list(range(D)),
        trace=_trace,
        **(_trace_kwargs or {}),
    )
    out = np.zeros((N + 2 * E, W), np.float32)
    # kcl block: out[0:N, 0:E] = M verbatim (host-side copy of the input;
    # no device round-trip)
    out[0:N, 0:E] = M
    for d in range(D):
        r = res.results[d]
        # kvl -M^T block: column slice [E, 256] for this core's nodes.
        # mtc[p, g*4096 + cb*128 + j] = -M[128g+j, 128cb+p]
        v = np.asarray(r["mtc"]).reshape(128, 2, 32, 128)
        mts = v.transpose(2, 0, 1, 3).reshape(E, NR)
        out[N : N + E, 2 * E + d * NR : 2 * E + (d + 1) * NR] = mts
        # diagonals: zyo = [z | y | ones], value layout r = c*128 + p
        gs = d * EC + np.arange(EC)
        zy = r["zyo"]
        z_flat = zy[:, 0:4].T.reshape(EC)
        y_flat = zy[:, 4:8].T.reshape(EC)
        o_flat = zy[:, 8:12].T.reshape(EC)
        out[N + gs, E + gs] = o_flat  # I_E diag in kvl rows
        out[N + E + gs, gs] = z_flat  # elem z diag
        out[N + E + gs, E + gs] = y_flat  # elem y diag
    if _trace:
        _CACHE["last_result"] = res
    return out
